# revision 14
# baseline (speedup 1.0000x reference)
"""Trainium2 Bass kernel for a 1-layer LSTM encoder (batch=1, seq=1024).

Computation (matches the PyTorch/JAX reference):
    xs = emb[tokens]                       # [S, E] embedding gather
    x_gates = xs @ W_ih.T + (b_ih + b_hh)  # [S, 4H] bulk input projection
    per step t (PAD steps are identity and are skipped):
        g = x_gates[t] + W_hh @ h
        i, f, g', o = split(g); c = sig(f)*c + sig(i)*tanh(g'); h = sig(o)*tanh(c)
    returns (h_final, (h_final, c_final))  # out == h at last non-PAD step

Device strategy: the recurrence is strictly serial (1024 dependent matvecs
streaming all 4MB of W_hh through the PE each step), and cross-core
collectives have multi-microsecond floors, so each of the 8 NeuronCores runs
the full computation redundantly (identical SPMD program, replicated inputs);
core 0's output is used.  On-device phases per core:
  1. indirect-DMA gather of the 1024 embedding rows,
  2. PE transposes -> xs^T with a fused bias row,
  3. bulk matmul x_gates^T = W_ihb^T.T @ xs^T  (gates on partitions),
  4. 1024-step recurrence: 64 accumulating matvec matmuls per step into
     PSUM[128,16], one fused DVE add of the x_gates column, sigmoid/tanh on
     ScalarE, LSTM cell update on VectorE.
Gate tiles are host-permuted to [i, f, o, g] so one sigmoid covers i|f|o.
"""

import os
from contextlib import ExitStack

import numpy as np

import concourse.bass as bass
import concourse.tile as tile
from concourse import bacc, mybir
from concourse.bass_utils import run_bass_kernel_spmd
from concourse.masks import make_identity

VOCAB, EMBED, HIDDEN, SEQ = 100000, 300, 512, 1024
PAD_IDX = 1
P = 128
H4 = 4 * HIDDEN            # 2048 gate rows
NM = H4 // P               # 16 gate tiles (m)
NK = HIDDEN // P           # 4 contraction chunks (k)
N_CORES = 8
F32 = mybir.dt.float32

# gate blocks in PyTorch row order are [i, f, g, o]; we want tile order
# [i, f, o, g] so a single sigmoid covers tiles 0..11 and tanh covers 12..15
_GATE_PERM = np.concatenate([
    np.arange(0, 512), np.arange(512, 1024),
    np.arange(1536, 2048), np.arange(1024, 1536),
])

# weight dtype for the recurrent matvec ("f32" | "bf16"), overridable for tuning
W_DTYPE = os.environ.get("LSTM_W_DTYPE", "f32")
# elementwise mode: "v1" = sigmoid(12)+tanh(4); "v2" = one sigmoid(16) with
# host-prescaled g rows (tanh(x) = 2*sigmoid(2x)-1, fixed up by tensor_scalar)
ELT_MODE = os.environ.get("LSTM_ELT", "v1")

_cache: dict = {}


def _build(nonpad_steps: tuple, w_dtype: str, elt_mode: str):
    """Build + compile the SPMD program for the given non-PAD step list."""
    wdt = F32 if w_dtype == "f32" else mybir.dt.bfloat16
    nc = bacc.Bacc("TRN2", target_bir_lowering=False, debug=False,
                   num_devices=N_CORES)

    d_emb = nc.dram_tensor("emb", [VOCAB, EMBED], F32, kind="ExternalInput")
    d_tok = nc.dram_tensor("tok", [P, SEQ // P], mybir.dt.int32,
                           kind="ExternalInput")
    d_wih0 = nc.dram_tensor("wih0", [P, H4], F32, kind="ExternalInput")
    d_wih1 = nc.dram_tensor("wih1", [P, H4], F32, kind="ExternalInput")
    d_wih2 = nc.dram_tensor("wih2", [44, H4], F32, kind="ExternalInput")
    d_bias = nc.dram_tensor("bias", [P, NM], F32, kind="ExternalInput")
    d_whh = nc.dram_tensor("whh", [P, NM * NK * P], wdt, kind="ExternalInput")
    d_hc0 = nc.dram_tensor("hc0", [P, 8], F32, kind="ExternalInput")
    d_out = nc.dram_tensor("hc_out", [P, 8], F32, kind="ExternalOutput")

    with tile.TileContext(nc) as tc, ExitStack() as ctx:
        const = ctx.enter_context(tc.tile_pool(name="const", bufs=1))
        work = ctx.enter_context(tc.tile_pool(name="work", bufs=3))
        spool = ctx.enter_context(tc.tile_pool(name="spool", bufs=2))
        psum = ctx.enter_context(tc.tile_pool(name="psum", bufs=2, space="PSUM"))
        psg = ctx.enter_context(tc.tile_pool(name="psg", bufs=2, space="PSUM"))

        # ---- persistent SBUF state ----
        whh_sb = const.tile([P, NM * NK * P], wdt)
        wih0_sb = const.tile([P, H4], F32)
        wih1_sb = const.tile([P, H4], F32)
        wih2_sb = const.tile([44, H4], F32)
        bias_sb = const.tile([P, NM], F32)
        tok_sb = const.tile([P, SEQ // P], mybir.dt.int32)
        hc0_sb = const.tile([P, 8], F32)
        ident = const.tile([P, P], F32)
        xsT0 = const.tile([P, SEQ], F32)
        xsT1 = const.tile([P, SEQ], F32)
        xsT2 = const.tile([44, SEQ], F32)
        xgT = const.tile([P, NM * SEQ], F32)      # tile-major: m*SEQ + t
        h_t = const.tile([P, NK], wdt)
        c_t = const.tile([P, NK], F32)

        nc.sync.dma_start(out=whh_sb[:], in_=d_whh[:, :])
        nc.sync.dma_start(out=wih0_sb[:], in_=d_wih0[:, :])
        nc.sync.dma_start(out=wih1_sb[:], in_=d_wih1[:, :])
        nc.sync.dma_start(out=wih2_sb[:], in_=d_wih2[:, :])
        nc.sync.dma_start(out=bias_sb[:], in_=d_bias[:, :])
        nc.sync.dma_start(out=tok_sb[:], in_=d_tok[:, :])
        nc.sync.dma_start(out=hc0_sb[:], in_=d_hc0[:, :])
        make_identity(nc, ident[:])

        # ---- phase 1+2: gather embedding rows, transpose to xs^T ----
        echunks = [(0, 128, xsT0), (128, 128, xsT1), (256, 44, xsT2)]
        for j in range(SEQ // P):
            xs_j = work.tile([P, EMBED], F32)
            nc.gpsimd.indirect_dma_start(
                out=xs_j[:], out_offset=None, in_=d_emb[:, :],
                in_offset=bass.IndirectOffsetOnAxis(ap=tok_sb[:, j:j + 1], axis=0),
            )
            for (e0, esz, dstT) in echunks:
                pt = psum.tile([P, P], F32)
                nc.tensor.transpose(out=pt[:esz, :P], in_=xs_j[:, e0:e0 + esz],
                                    identity=ident[:])
                nc.vector.tensor_copy(dstT[:esz, j * P:(j + 1) * P], pt[:esz, :P])

        # ---- phase 3: x_gates^T = W_ih^T.T @ xs^T + b (b fused in copy) ----
        kchunks = [(128, wih0_sb, xsT0), (128, wih1_sb, xsT1), (44, wih2_sb, xsT2)]
        for m in range(NM):
            for nchunk in range(SEQ // 512):
                pg = psum.tile([P, 512], F32)
                for ki, (ksz, wih_k, xsT_k) in enumerate(kchunks):
                    nc.tensor.matmul(
                        pg[:, :],
                        wih_k[:ksz, m * P:(m + 1) * P],
                        xsT_k[:ksz, nchunk * 512:(nchunk + 1) * 512],
                        start=(ki == 0), stop=(ki == 2),
                    )
                nc.vector.tensor_scalar(
                    xgT[:, m * SEQ + nchunk * 512: m * SEQ + (nchunk + 1) * 512],
                    pg[:, :], bias_sb[:, m:m + 1], None, mybir.AluOpType.add)

        xgTv = xgT[:].rearrange("p (g t) -> p g t", t=SEQ)

        # ---- init h, c ----
        nc.vector.tensor_copy(h_t[:], hc0_sb[:, 0:4])
        nc.vector.tensor_copy(c_t[:], hc0_sb[:, 4:8])

        # ---- phase 4: the recurrence ----
        for t in nonpad_steps:
            pgate = psg.tile([P, NM], F32)
            for m in range(NM):
                for k in range(NK):
                    blk = (m * NK + k) * P
                    nc.tensor.matmul(
                        pgate[:, m:m + 1],
                        whh_sb[:, blk:blk + P],
                        h_t[:, k:k + 1],
                        start=(k == 0), stop=(k == NK - 1),
                    )
            g_sb = spool.tile([P, NM], F32)
            nc.vector.tensor_add(g_sb[:], pgate[:], xgTv[:, :, t:t + 1])
            if elt_mode == "v1":
                sig = spool.tile([P, 12], F32)
                nc.scalar.activation(sig[:], g_sb[:, 0:12],
                                     mybir.ActivationFunctionType.Sigmoid)
                gt = spool.tile([P, 4], F32)
                nc.scalar.activation(gt[:], g_sb[:, 12:16],
                                     mybir.ActivationFunctionType.Tanh)
            else:
                # g rows were pre-scaled by 2 on host: tanh(x) = 2*sig(2x)-1
                sig = spool.tile([P, NM], F32)
                nc.scalar.activation(sig[:], g_sb[:],
                                     mybir.ActivationFunctionType.Sigmoid)
                gt = spool.tile([P, 4], F32)
                nc.vector.tensor_scalar(gt[:], sig[:, 12:16], 2.0, -1.0,
                                        mybir.AluOpType.mult,
                                        mybir.AluOpType.add)
            m2 = spool.tile([P, 4], F32)
            nc.vector.tensor_mul(m2[:], sig[:, 4:8], c_t[:])
            m1 = spool.tile([P, 4], F32)
            nc.vector.tensor_mul(m1[:], sig[:, 0:4], gt[:])
            nc.vector.tensor_add(c_t[:], m1[:], m2[:])
            tch = spool.tile([P, 4], F32)
            nc.scalar.activation(tch[:], c_t[:],
                                 mybir.ActivationFunctionType.Tanh)
            nc.vector.tensor_mul(h_t[:], sig[:, 8:12], tch[:])

        # ---- store final h (cols 0:4) and c (cols 4:8) ----
        if wdt is F32:
            h_out = h_t
        else:
            h_out = const.tile([P, NK], F32)
            nc.vector.tensor_copy(h_out[:], h_t[:])
        nc.sync.dma_start(out=d_out[:, 0:4], in_=h_out[:])
        nc.sync.dma_start(out=d_out[:, 4:8], in_=c_t[:])

    nc.compile()
    return nc


def prep_inputs(input_seq, h0, c0, emb, W_ih, W_hh, b_ih, b_hh,
                w_dtype=None, elt_mode=None):
    """Host-side input prep: layouts, permutations, dtypes. Returns
    (in_map, nonpad_steps) or (None, None) if every token is PAD."""
    w_dtype = w_dtype or W_DTYPE
    elt_mode = elt_mode or ELT_MODE
    tokens = np.asarray(input_seq).reshape(-1).astype(np.int64)
    nonpad = np.nonzero(tokens != PAD_IDX)[0]
    if len(nonpad) == 0:
        return None, None

    emb = np.ascontiguousarray(np.asarray(emb, dtype=np.float32))
    tok = np.ascontiguousarray(
        tokens.astype(np.int32).reshape(SEQ // P, P).T)      # [128, 8]

    Wp = np.asarray(W_ih, dtype=np.float32)[_GATE_PERM].copy()  # [2048, 300]
    bp = ((np.asarray(b_ih, dtype=np.float32)
           + np.asarray(b_hh, dtype=np.float32))[_GATE_PERM]).copy()  # [2048]
    Whp = np.asarray(W_hh, dtype=np.float32)[_GATE_PERM].copy()  # [2048, 512]
    if elt_mode == "v2":
        # pre-scale the g-gate rows (tiles 12..15) by 2: tanh(x)=2*sig(2x)-1
        Wp[1536:] *= 2.0
        bp[1536:] *= 2.0
        Whp[1536:] *= 2.0

    wihT = Wp.T                                               # [300, 2048]
    wih0 = np.ascontiguousarray(wihT[0:128])
    wih1 = np.ascontiguousarray(wihT[128:256])
    wih2 = np.ascontiguousarray(wihT[256:300])                # [44, 2048]
    bias = np.ascontiguousarray(bp.reshape(NM, P).T)          # [128, 16]

    # whh_sb[p, (m*NK+k)*128 + q] = Whp[m*128+q, k*128+p]
    whh = np.transpose(Whp.reshape(NM, P, NK, P), (3, 0, 2, 1)).reshape(P, -1)
    whh = np.ascontiguousarray(
        whh if w_dtype == "f32" else whh.astype(np.dtype("bfloat16")))

    h0f = np.asarray(h0, dtype=np.float32).reshape(HIDDEN)
    c0f = np.asarray(c0, dtype=np.float32).reshape(HIDDEN)
    hc0 = np.ascontiguousarray(
        np.concatenate([h0f.reshape(NK, P).T, c0f.reshape(NK, P).T], axis=1))

    in_map = {"emb": emb, "tok": tok, "wih0": wih0, "wih1": wih1,
              "wih2": wih2, "bias": bias, "whh": whh, "hc0": hc0}
    return in_map, tuple(int(t) for t in nonpad)


def get_nc(nonpad_steps, w_dtype=None, elt_mode=None):
    w_dtype = w_dtype or W_DTYPE
    elt_mode = elt_mode or ELT_MODE
    key = (nonpad_steps, w_dtype, elt_mode)
    if key not in _cache:
        _cache[key] = _build(nonpad_steps, w_dtype, elt_mode)
    return _cache[key]


def postprocess(hc_out):
    h = np.ascontiguousarray(hc_out[:, 0:4].T.reshape(HIDDEN))
    c = np.ascontiguousarray(hc_out[:, 4:8].T.reshape(HIDDEN))
    return (h[None, None, :], (h[None, None, :].copy(), c[None, None, :]))


def kernel(input_seq, h0, c0, emb, W_ih, W_hh, b_ih, b_hh):
    in_map, nonpad = prep_inputs(input_seq, h0, c0, emb, W_ih, W_hh, b_ih, b_hh)
    if in_map is None:
        # every token is PAD: out stays zeros, state stays (h0, c0)
        z = np.zeros((1, 1, HIDDEN), np.float32)
        return (z, (np.asarray(h0, np.float32).reshape(1, 1, HIDDEN).copy(),
                    np.asarray(c0, np.float32).reshape(1, 1, HIDDEN).copy()))
    nc = get_nc(nonpad)
    res = run_bass_kernel_spmd(nc, [in_map] * N_CORES,
                               core_ids=list(range(N_CORES)))
    return postprocess(res.results[0]["hc_out"])
